# revision 6
# baseline (speedup 1.0000x reference)
"""CASSI colored-aperture layer (nn_CASSI_layer_Colored) on 8 Trainium2 NeuronCores.

Reference semantics (B=4, M=N=KERN=256, L=24 bands, S=22 shots):
    H[m,n,l,s] = (wr*fr[l] + wg*fg[l] + wb*fb[l] + wc*fc[l]) / (wr+wg+wb+wc)
    Y[b,m,n',s] = sum_l H[m,n'-l,l,s] * x[b,m,n'-l,l]          (dispersion shift-sum)
    X[b,m,n,l]  = sum_s H[m,n,l,s] * Y[b,m,n+l,s]              (adjoint + shot sum)
    out = X / max(X)

Sharding: data-parallel over (batch b, row-half mh): 4 x 2 = 8 cores.  Rows m
never couple, so each core computes 128 rows of one batch independently; only
the final global max couples shards (done on host after the gather).

On-core layout: partitions = 128 m-rows, free dims = (n, s) or (n, l); the
dispersion shifts n -> n+l become plain free-dim address offsets.  Everything
runs in fp16 (validated ~7e-4 max rel err vs fp64) on the Vector engine with
fused scalar_tensor_tensor ops, with ScalarE doing the f[l]-scaled seeds and
the x broadcast-over-s copies, and GpSimd one fused op per l plus the Y memset.
"""

import numpy as np

B, M, N, L, S = 4, 256, 256, 24, 22
MSH = M // 2                     # rows per core
NCORES = 8
NS, NL = N * S, N * L
YW = (N + L - 1) * S             # Y free width: 279 * 22


def _bases() -> np.ndarray:
    """(4, L) color responses paired row-wise with (wr, wg, wb, wc)."""
    wl = np.linspace(400.0, 700.0, L)

    def g(mu: float, sig: float) -> np.ndarray:
        return np.exp(-0.5 * ((wl - mu) / sig) ** 2)

    # reference: H = wr*f620 + wg*f550 + wb*f450 + wc*f500 (fr,fg,fc,fb = 620,550,500,450)
    return np.stack([g(620.0, 50.0), g(550.0, 50.0), g(450.0, 50.0), g(500.0, 50.0)])


_NC = None


def _build():
    import concourse.bacc as bacc
    import concourse.mybir as mybir
    import concourse.tile as tile

    f16, f32 = mybir.dt.float16, mybir.dt.float32
    A = mybir.AluOpType
    F = _bases()

    nc = bacc.Bacc("TRN2", target_bir_lowering=False, debug=False, num_devices=NCORES)
    xin = nc.declare_dram_parameter("x16", [MSH, NL], f16, isOutput=False)
    wins = [
        nc.declare_dram_parameter(f"w{i}", [MSH, NS], f16, isOutput=False)
        for i in range(4)
    ]
    out = nc.declare_dram_parameter("out", [MSH, NL], f32, isOutput=True)

    with tile.TileContext(nc) as tc:
        with (
            tc.tile_pool(name="main", bufs=1) as main,
            tc.tile_pool(name="hp", bufs=2) as hp,
            tc.tile_pool(name="pp", bufs=2) as pp,
        ):
            a = [main.tile([MSH, NS], f16, tag=f"a{i}", name=f"a{i}") for i in range(4)]
            xt = main.tile([MSH, NL], f16, tag="x", name="xt")
            Y = main.tile([MSH, YW], f16, tag="Y", name="Yt")
            Xo = main.tile([MSH, NL], f32, tag="Xo", name="Xot")
            u = hp.tile([MSH, NS], f16, tag="h", name="ut")

            for i in range(4):
                nc.sync.dma_start(a[i][:], wins[i][:])
            nc.sync.dma_start(xt[:], xin[:])
            nc.gpsimd.memset(Y[:], 0.0)

            # a_c = w_c / (wr+wg+wb+wc)
            nc.vector.tensor_tensor(u[:], a[0][:], a[1][:], A.add)
            nc.vector.tensor_tensor(u[:], u[:], a[2][:], A.add)
            nc.vector.tensor_tensor(u[:], u[:], a[3][:], A.add)
            with nc.allow_low_precision("fp16 pipeline, validated 7e-4 vs fp64"):
                nc.vector.reciprocal(u[:], u[:])
            for i in range(4):
                nc.vector.tensor_tensor(a[i][:], a[i][:], u[:], A.mult)

            x3 = xt[:].rearrange("p (n l) -> p n l", l=L)

            def build_h(l: int):
                """h[m,n,s] = sum_c F[c,l] * a_c[m,n,s] across ScalarE/DVE/GpSimd."""
                h = hp.tile([MSH, NS], f16, tag="h", name="ht")
                t3 = hp.tile([MSH, NS], f16, tag="t3", name="t3t")
                nc.scalar.mul(h[:], a[0][:], float(F[0, l]))
                nc.vector.scalar_tensor_tensor(
                    h[:], a[1][:], float(F[1, l]), h[:], A.mult, A.add
                )
                nc.vector.scalar_tensor_tensor(
                    h[:], a[2][:], float(F[2, l]), h[:], A.mult, A.add
                )
                nc.scalar.mul(t3[:], a[3][:], float(F[3, l]))
                nc.gpsimd.tensor_tensor(h[:], h[:], t3[:], A.add)
                return h

            # Stage 1: Y[m, n+l, s] += h_l[m, n, s] * x[m, n, l]
            for l in range(L):
                h = build_h(l)
                xb = pp.tile([MSH, NS], f16, tag="xb", name="xbt")
                src = x3[:, :, l].unsqueeze(2).broadcast_to((MSH, N, S))
                nc.scalar.copy(xb[:].rearrange("p (n s) -> p n s", s=S), src)
                p = pp.tile([MSH, NS], f16, tag="p", name="pt")
                nc.vector.tensor_tensor(p[:], h[:], xb[:], A.mult)
                ysl = Y[:, l * S : l * S + NS]
                nc.vector.tensor_tensor(ysl, ysl, p[:], A.add)

            # Stage 2: X[m, n, l] = sum_s h_l[m, n, s] * Y[m, n+l, s]
            X3 = Xo[:].rearrange("p (n l) -> p n l", l=L)
            for l in range(L):
                h = build_h(l)
                t = pp.tile([MSH, NS], f16, tag="p", name="pt")
                nc.vector.tensor_tensor(t[:], h[:], Y[:, l * S : l * S + NS], A.mult)
                nc.vector.tensor_reduce(
                    X3[:, :, l],
                    t[:].rearrange("p (n s) -> p n s", s=S),
                    axis=mybir.AxisListType.X,
                    op=A.add,
                )

            nc.sync.dma_start(out[:], Xo[:])

    nc.compile()
    return nc


def _get_nc():
    global _NC
    if _NC is None:
        _NC = _build()
    return _NC


def _make_in_maps(x, wr, wg, wb, wc):
    x = np.asarray(x, dtype=np.float32)
    ws = [np.asarray(w, dtype=np.float32).reshape(M, M, S) for w in (wr, wg, wb, wc)]
    in_maps = []
    for core in range(NCORES):
        b, mh = divmod(core, 2)
        rows = slice(mh * MSH, (mh + 1) * MSH)
        m = {"x16": np.ascontiguousarray(x[b, rows]).reshape(MSH, NL).astype(np.float16)}
        for i, w in enumerate(ws):
            m[f"w{i}"] = np.ascontiguousarray(w[rows]).reshape(MSH, NS).astype(np.float16)
        in_maps.append(m)
    return in_maps


def _run_shards(in_maps):
    from concourse.bass_utils import run_bass_kernel_spmd

    nc = _get_nc()
    return run_bass_kernel_spmd(nc, in_maps, list(range(NCORES)))


def kernel(x, wr, wg, wb, wc):
    res = _run_shards(_make_in_maps(x, wr, wg, wb, wc))
    X = np.empty((B, M, N, L), dtype=np.float32)
    for core in range(NCORES):
        b, mh = divmod(core, 2)
        X[b, mh * MSH : (mh + 1) * MSH] = res.results[core]["out"].reshape(MSH, N, L)
    return X / X.max()


def estimate_ns() -> float:
    """Single-core cost-model estimate of the kernel duration (ns)."""
    from concourse.timeline_sim import TimelineSim

    return TimelineSim(_get_nc()).simulate()


# revision 7
# speedup vs baseline: 1.7986x; 1.7986x over previous
"""CASSI colored-aperture layer (nn_CASSI_layer_Colored) on 8 Trainium2 NeuronCores.

Reference semantics (B=4, M=N=KERN=256, L=24 bands, S=22 shots):
    H[m,n,l,s] = (wr*fr[l] + wg*fg[l] + wb*fb[l] + wc*fc[l]) / (wr+wg+wb+wc)
    Y[b,m,n',s] = sum_l H[m,n'-l,l,s] * x[b,m,n'-l,l]          (dispersion shift-sum)
    X[b,m,n,l]  = sum_s H[m,n,l,s] * Y[b,m,n+l,s]              (adjoint + shot sum)
    out = X / max(X)

Sharding: data-parallel over (batch b, row-half mh): 4 x 2 = 8 cores.  Rows m
never couple, so each core computes 128 rows of one batch independently; only
the final global max couples shards (host side, after the gather).

Per-core mapping: partitions = 128 m-rows; free dims are s-major (s, n) so the
dispersion shift n -> n+l is a free-dim offset, the broadcast of x over s is a
stride-0 outer AP dim (dense innermost keeps DVE 2x mode), and the shot-sum
becomes contiguous stripe-halving adds.  Pipeline is fp16 (~1e-3 max rel err
vs fp64, validated).  Per band l:
  stage 1: h_l = sum_c F[c,l]*a_c (ScalarE seeds + partials, DVE/GpSimd adds),
           Y[:, l:l+N] += h_l * x[:, l-bcast]  (DVE), h_l spilled to DRAM
  stage 2: h_l reloaded (DMA, hidden), t = h_l * Y[:, l:l+N] (DVE),
           X[:, l] = stripe-tree shot sum (GpSimd first level, DVE rest)
"""

import numpy as np

B, M, N, L, S = 4, 256, 256, 24, 22
MSH = M // 2                     # rows per core
NCORES = 8
NS, NL = N * S, N * L
NP = N + L - 1                   # 279 shifted columns
YW = NP * S                      # Y free width (s-major: s outer, n' inner)


def _bases() -> np.ndarray:
    """(4, L) color responses paired row-wise with (wr, wg, wb, wc)."""
    wl = np.linspace(400.0, 700.0, L)

    def g(mu: float, sig: float) -> np.ndarray:
        return np.exp(-0.5 * ((wl - mu) / sig) ** 2)

    # reference: H = wr*f620 + wg*f550 + wb*f450 + wc*f500 (fr,fg,fc,fb = 620,550,500,450)
    return np.stack([g(620.0, 50.0), g(550.0, 50.0), g(450.0, 50.0), g(500.0, 50.0)])


_NC = None


def _build():
    import concourse.bacc as bacc
    import concourse.mybir as mybir
    import concourse.tile as tile

    f16, f32 = mybir.dt.float16, mybir.dt.float32
    A = mybir.AluOpType
    F = _bases()

    nc = bacc.Bacc("TRN2", target_bir_lowering=False, debug=False, num_devices=NCORES)
    xin = nc.declare_dram_parameter("x16", [MSH, NL], f16, isOutput=False)   # (l, n)
    wins = [
        nc.declare_dram_parameter(f"w{i}", [MSH, NS], f16, isOutput=False)   # (s, n)
        for i in range(4)
    ]
    out = nc.declare_dram_parameter("out", [MSH, NL], f32, isOutput=True)    # (l, n)
    hcache = nc.dram_tensor("hcache", [L, MSH, NS], f16)

    with tile.TileContext(nc) as tc:
        with (
            tc.tile_pool(name="main", bufs=1) as main,
            tc.tile_pool(name="hp", bufs=3) as hp,
            tc.tile_pool(name="tp", bufs=4) as tp,
            tc.tile_pool(name="pp", bufs=2) as pp,
        ):
            a = [main.tile([MSH, NS], f16, tag=f"a{i}", name=f"a{i}") for i in range(4)]
            xt = main.tile([MSH, NL], f16, tag="x", bufs=2, name="xt")
            Y = main.tile([MSH, YW], f16, tag="Y", name="Yt")
            Xo = main.tile([MSH, NL], f32, tag="Xo", name="Xot")

            for i in range(4):
                nc.sync.dma_start(a[i][:], wins[i][:])
            nc.sync.dma_start(xt[:], xin[:])
            nc.gpsimd.memset(Y[:], 0.0)

            # a_c = w_c / (wr+wg+wb+wc)
            u = hp.tile([MSH, NS], f16, tag="h", name="ut")
            nc.vector.tensor_tensor(u[:], a[0][:], a[1][:], A.add)
            nc.vector.tensor_tensor(u[:], u[:], a[2][:], A.add)
            nc.vector.tensor_tensor(u[:], u[:], a[3][:], A.add)
            with nc.allow_low_precision("fp16 pipeline, validated ~1e-3 vs fp64"):
                nc.vector.reciprocal(u[:], u[:])
            for i in range(4):
                nc.vector.tensor_tensor(a[i][:], a[i][:], u[:], A.mult)

            x3 = xt[:].rearrange("p (l n) -> p l n", n=N)
            Y3 = Y[:].rearrange("p (s n) -> p s n", n=NP)

            # Stage 1: Y[:, s, l+n] += h_l[:, s, n] * x[:, l, n];  h_l -> DRAM
            for l in range(L):
                h = hp.tile([MSH, NS], f16, tag="h", name="ht")
                t1 = tp.tile([MSH, NS], f16, tag="tp", name="t1t")
                t2 = tp.tile([MSH, NS], f16, tag="tp", name="t2t")
                t3 = tp.tile([MSH, NS], f16, tag="tp", name="t3t")
                nc.scalar.mul(h[:], a[0][:], float(F[0, l]))          # ACT seed
                nc.vector.tensor_scalar_mul(t1[:], a[1][:], float(F[1, l]))
                nc.scalar.mul(t2[:], a[2][:], float(F[2, l]))         # ACT partial
                nc.vector.tensor_scalar_mul(t3[:], a[3][:], float(F[3, l]))
                nc.vector.tensor_tensor(h[:], h[:], t1[:], A.add)
                nc.vector.tensor_tensor(h[:], h[:], t2[:], A.add)
                nc.gpsimd.tensor_tensor(h[:], h[:], t3[:], A.add)     # GpSimd add
                nc.sync.dma_start(hcache[l], h[:])
                p = pp.tile([MSH, NS], f16, tag="p", name="pt")
                xb = x3[:, l, :].unsqueeze(1).broadcast_to((MSH, S, N))
                nc.vector.tensor_tensor(
                    p[:].rearrange("p (s n) -> p s n", n=N),
                    h[:].rearrange("p (s n) -> p s n", n=N),
                    xb,
                    A.mult,
                )
                ysl = Y3[:, :, l : l + N]
                nc.vector.tensor_tensor(
                    ysl, ysl, p[:].rearrange("p (s n) -> p s n", n=N), A.add
                )

            # Stage 2: X[:, l, n] = sum_s h_l[:, s, n] * Y[:, s, l+n]
            for l in range(L):
                h = main.tile([MSH, NL], f16, tag="x", bufs=2, name="hin")
                nc.sync.dma_start(h[:, :NS], hcache[l])
                t = pp.tile([MSH, NS], f16, tag="p", name="tt")
                nc.vector.tensor_tensor(
                    t[:].rearrange("p (s n) -> p s n", n=N),
                    h[:, :NS].rearrange("p (s n) -> p s n", n=N),
                    Y3[:, :, l : l + N],
                    A.mult,
                )
                # shot-sum tree over 22 contiguous stripes of N
                tv = t[:]
                nc.gpsimd.tensor_tensor(
                    tv[:, : 11 * N], tv[:, : 11 * N], tv[:, 11 * N : 22 * N], A.add
                )
                nc.vector.tensor_tensor(
                    tv[:, : 5 * N], tv[:, : 5 * N], tv[:, 5 * N : 10 * N], A.add
                )
                nc.vector.tensor_tensor(
                    tv[:, : 2 * N], tv[:, : 2 * N], tv[:, 2 * N : 4 * N], A.add
                )
                nc.vector.tensor_tensor(tv[:, :N], tv[:, :N], tv[:, N : 2 * N], A.add)
                nc.vector.tensor_tensor(
                    tv[:, :N], tv[:, :N], tv[:, 4 * N : 5 * N], A.add
                )
                nc.vector.tensor_tensor(
                    Xo[:, l * N : (l + 1) * N], tv[:, :N], tv[:, 10 * N : 11 * N], A.add
                )

            nc.sync.dma_start(out[:], Xo[:])

    nc.compile()
    return nc


def _get_nc():
    global _NC
    if _NC is None:
        _NC = _build()
    return _NC


def _make_in_maps(x, wr, wg, wb, wc):
    x = np.asarray(x, dtype=np.float32)
    ws = [np.asarray(w, dtype=np.float32).reshape(M, M, S) for w in (wr, wg, wb, wc)]
    in_maps = []
    for core in range(NCORES):
        b, mh = divmod(core, 2)
        rows = slice(mh * MSH, (mh + 1) * MSH)
        xs = x[b, rows].transpose(0, 2, 1)            # (MSH, L, N)
        m = {"x16": np.ascontiguousarray(xs).reshape(MSH, NL).astype(np.float16)}
        for i, w in enumerate(ws):
            wsb = w[rows].transpose(0, 2, 1)          # (MSH, S, N)
            m[f"w{i}"] = np.ascontiguousarray(wsb).reshape(MSH, NS).astype(np.float16)
        in_maps.append(m)
    return in_maps


def _run_shards(in_maps):
    from concourse.bass_utils import run_bass_kernel_spmd

    nc = _get_nc()
    return run_bass_kernel_spmd(nc, in_maps, list(range(NCORES)))


def kernel(x, wr, wg, wb, wc):
    res = _run_shards(_make_in_maps(x, wr, wg, wb, wc))
    X = np.empty((B, M, N, L), dtype=np.float32)
    for core in range(NCORES):
        b, mh = divmod(core, 2)
        xo = res.results[core]["out"].reshape(MSH, L, N).transpose(0, 2, 1)
        X[b, mh * MSH : (mh + 1) * MSH] = xo
    return X / X.max()


def estimate_ns() -> float:
    """Single-core cost-model estimate of the kernel duration (ns)."""
    from concourse.timeline_sim import TimelineSim

    return TimelineSim(_get_nc()).simulate()


# revision 10
# speedup vs baseline: 2.1370x; 1.1881x over previous
"""CASSI colored-aperture layer (nn_CASSI_layer_Colored) on 8 Trainium2 NeuronCores.

Reference semantics (B=4, M=N=KERN=256, L=24 bands, S=22 shots):
    H[m,n,l,s] = (wr*fr[l] + wg*fg[l] + wb*fb[l] + wc*fc[l]) / (wr+wg+wb+wc)
    Y[b,m,n',s] = sum_l H[m,n'-l,l,s] * x[b,m,n'-l,l]          (dispersion shift-sum)
    X[b,m,n,l]  = sum_s H[m,n,l,s] * Y[b,m,n+l,s]              (adjoint + shot sum)
    out = X / max(X)

Sharding: data-parallel over (batch b, row-half mh): 4 x 2 = 8 cores.  Rows m
never couple, so each core computes 128 rows of one batch independently; only
the final global max couples shards (host side, after the gather).

Per-core mapping: partitions = 128 m-rows; free dims are s-major (s, n) so the
dispersion shift n -> n+l is a free-dim offset, the broadcast of x over s is a
stride-0 outer AP dim (dense innermost keeps DVE 2x mode), and the shot-sum
becomes contiguous stripe-halving adds.  Pipeline is fp16 (~1e-3 max rel err
vs fp64, validated).  Per band l:
  stage 1: h_l = sum_c F[c,l]*a_c (ScalarE seeds + partials, DVE/GpSimd adds),
           Y[:, l:l+N] += h_l * x[:, l-bcast]  (DVE), h_l spilled to DRAM
  stage 2: h_l reloaded (DMA, hidden), t = h_l * Y[:, l:l+N] (DVE),
           X[:, l] = stripe-tree shot sum (GpSimd first level, DVE rest)
"""

import numpy as np

B, M, N, L, S = 4, 256, 256, 24, 22
MSH = M // 2                     # rows per core
NCORES = 8
NS, NL = N * S, N * L
NP = N + L - 1                   # 279 shifted columns
YW = NP * S                      # Y free width (s-major: s outer, n' inner)


def _bases() -> np.ndarray:
    """(4, L) color responses paired row-wise with (wr, wg, wb, wc)."""
    wl = np.linspace(400.0, 700.0, L)

    def g(mu: float, sig: float) -> np.ndarray:
        return np.exp(-0.5 * ((wl - mu) / sig) ** 2)

    # reference: H = wr*f620 + wg*f550 + wb*f450 + wc*f500 (fr,fg,fc,fb = 620,550,500,450)
    return np.stack([g(620.0, 50.0), g(550.0, 50.0), g(450.0, 50.0), g(500.0, 50.0)])


_NC = None


def _build():
    import concourse.bacc as bacc
    import concourse.mybir as mybir
    import concourse.tile as tile

    f16, f32 = mybir.dt.float16, mybir.dt.float32
    A = mybir.AluOpType
    F = _bases()

    nc = bacc.Bacc("TRN2", target_bir_lowering=False, debug=False, num_devices=NCORES)
    xin = nc.declare_dram_parameter("x16", [MSH, NL], f16, isOutput=False)   # (l, n)
    wins = [
        nc.declare_dram_parameter(f"w{i}", [MSH, NS], f16, isOutput=False)   # (s, n)
        for i in range(4)
    ]
    out = nc.declare_dram_parameter("out", [MSH, NL], f32, isOutput=True)    # (l, n)
    hcache = nc.dram_tensor("hcache", [L, MSH, NS], f16)

    with tile.TileContext(nc) as tc:
        with (
            tc.tile_pool(name="main", bufs=1) as main,
            tc.tile_pool(name="hp", bufs=3) as hp,
            tc.tile_pool(name="tp", bufs=4) as tp,
            tc.tile_pool(name="pp", bufs=2) as pp,
        ):
            a = [main.tile([MSH, NS], f16, tag=f"a{i}", name=f"a{i}") for i in range(4)]
            xt = main.tile([MSH, NL], f16, tag="x", bufs=2, name="xt")
            Y = main.tile([MSH, YW], f16, tag="Y", name="Yt")

            for i in range(4):
                nc.sync.dma_start(a[i][:], wins[i][:])
            nc.sync.dma_start(xt[:], xin[:])
            nc.gpsimd.memset(Y[:], 0.0)

            # a_c = w_c / (wr+wg+wb+wc)
            u = hp.tile([MSH, NS], f16, tag="h", name="ut")
            nc.vector.tensor_tensor(u[:], a[0][:], a[1][:], A.add)
            nc.vector.tensor_tensor(u[:], u[:], a[2][:], A.add)
            nc.vector.tensor_tensor(u[:], u[:], a[3][:], A.add)
            with nc.allow_low_precision("fp16 pipeline, validated ~1e-3 vs fp64"):
                nc.vector.reciprocal(u[:], u[:])
            for i in range(4):
                nc.vector.tensor_tensor(a[i][:], a[i][:], u[:], A.mult)

            x3 = xt[:].rearrange("p (l n) -> p l n", n=N)
            Y3 = Y[:].rearrange("p (s n) -> p s n", n=NP)

            # Stage 1: Y[:, s, l+n] += h_l[:, s, n] * x[:, l, n];  h_l -> DRAM
            for l in range(L):
                h = hp.tile([MSH, NS], f16, tag="h", name="ht")
                t1 = tp.tile([MSH, NS], f16, tag="tp", name="t1t")
                t2 = tp.tile([MSH, NS], f16, tag="tp", name="t2t")
                t3 = tp.tile([MSH, NS], f16, tag="tp", name="t3t")
                nc.scalar.mul(h[:], a[0][:], float(F[0, l]))          # ACT seed
                nc.vector.tensor_scalar_mul(t1[:], a[1][:], float(F[1, l]))
                nc.scalar.mul(t2[:], a[2][:], float(F[2, l]))         # ACT partial
                nc.scalar.mul(t3[:], a[3][:], float(F[3, l]))         # ACT partial
                nc.vector.tensor_tensor(h[:], h[:], t1[:], A.add)
                nc.vector.tensor_tensor(h[:], h[:], t2[:], A.add)
                nc.vector.tensor_tensor(h[:], h[:], t3[:], A.add)
                nc.sync.dma_start(hcache[l], h[:])
                p = pp.tile([MSH, NS], f16, tag="p", name="pt")
                xb = x3[:, l, :].unsqueeze(1).broadcast_to((MSH, S, N))
                nc.vector.tensor_tensor(
                    p[:].rearrange("p (s n) -> p s n", n=N),
                    h[:].rearrange("p (s n) -> p s n", n=N),
                    xb,
                    A.mult,
                )
                ysl = Y3[:, :, l : l + N]
                nc.gpsimd.tensor_tensor(                              # GpSimd acc
                    ysl, ysl, p[:].rearrange("p (s n) -> p s n", n=N), A.add
                )

            # Stage 2: X[:, l, n] = sum_s h_l[:, s, n] * Y[:, s, l+n]
            for l in range(L):
                h = main.tile([MSH, NL], f16, tag="x", bufs=2, name="hin")
                nc.sync.dma_start(h[:, :NS], hcache[l])
                t = pp.tile([MSH, NS], f16, tag="p", name="tt")
                nc.vector.tensor_tensor(
                    t[:].rearrange("p (s n) -> p s n", n=N),
                    h[:, :NS].rearrange("p (s n) -> p s n", n=N),
                    Y3[:, :, l : l + N],
                    A.mult,
                )
                # shot-sum tree over 22 contiguous stripes of N
                tv = t[:]
                nc.gpsimd.tensor_tensor(
                    tv[:, : 11 * N], tv[:, : 11 * N], tv[:, 11 * N : 22 * N], A.add
                )
                nc.vector.tensor_tensor(
                    tv[:, : 5 * N], tv[:, : 5 * N], tv[:, 5 * N : 10 * N], A.add
                )
                nc.vector.tensor_tensor(
                    tv[:, : 2 * N], tv[:, : 2 * N], tv[:, 2 * N : 4 * N], A.add
                )
                nc.vector.tensor_tensor(tv[:, :N], tv[:, :N], tv[:, N : 2 * N], A.add)
                nc.vector.tensor_tensor(
                    tv[:, :N], tv[:, :N], tv[:, 4 * N : 5 * N], A.add
                )
                xol = tp.tile([MSH, N], f32, tag="xol", bufs=2, name="xolt")
                nc.vector.tensor_tensor(
                    xol[:], tv[:, :N], tv[:, 10 * N : 11 * N], A.add
                )
                nc.sync.dma_start(out[:, l * N : (l + 1) * N], xol[:])

    nc.compile()
    return nc


def _get_nc():
    global _NC
    if _NC is None:
        _NC = _build()
    return _NC


def _make_in_maps(x, wr, wg, wb, wc):
    x = np.asarray(x, dtype=np.float32)
    ws = [np.asarray(w, dtype=np.float32).reshape(M, M, S) for w in (wr, wg, wb, wc)]
    in_maps = []
    for core in range(NCORES):
        b, mh = divmod(core, 2)
        rows = slice(mh * MSH, (mh + 1) * MSH)
        xs = x[b, rows].transpose(0, 2, 1)            # (MSH, L, N)
        m = {"x16": np.ascontiguousarray(xs).reshape(MSH, NL).astype(np.float16)}
        for i, w in enumerate(ws):
            wsb = w[rows].transpose(0, 2, 1)          # (MSH, S, N)
            m[f"w{i}"] = np.ascontiguousarray(wsb).reshape(MSH, NS).astype(np.float16)
        in_maps.append(m)
    return in_maps


def _run_shards(in_maps):
    from concourse.bass_utils import run_bass_kernel_spmd

    nc = _get_nc()
    return run_bass_kernel_spmd(nc, in_maps, list(range(NCORES)))


def kernel(x, wr, wg, wb, wc):
    res = _run_shards(_make_in_maps(x, wr, wg, wb, wc))
    X = np.empty((B, M, N, L), dtype=np.float32)
    for core in range(NCORES):
        b, mh = divmod(core, 2)
        xo = res.results[core]["out"].reshape(MSH, L, N).transpose(0, 2, 1)
        X[b, mh * MSH : (mh + 1) * MSH] = xo
    return X / X.max()


def estimate_ns() -> float:
    """Single-core cost-model estimate of the kernel duration (ns)."""
    from concourse.timeline_sim import TimelineSim

    return TimelineSim(_get_nc()).simulate()


# revision 11
# speedup vs baseline: 2.1490x; 1.0056x over previous
"""CASSI colored-aperture layer (nn_CASSI_layer_Colored) on 8 Trainium2 NeuronCores.

Reference semantics (B=4, M=N=KERN=256, L=24 bands, S=22 shots):
    H[m,n,l,s] = (wr*fr[l] + wg*fg[l] + wb*fb[l] + wc*fc[l]) / (wr+wg+wb+wc)
    Y[b,m,n',s] = sum_l H[m,n'-l,l,s] * x[b,m,n'-l,l]          (dispersion shift-sum)
    X[b,m,n,l]  = sum_s H[m,n,l,s] * Y[b,m,n+l,s]              (adjoint + shot sum)
    out = X / max(X)

Sharding: data-parallel over (batch b, row-half mh): 4 x 2 = 8 cores.  Rows m
never couple, so each core computes 128 rows of one batch independently; only
the final global max couples shards (host side, after the gather).

Per-core mapping: partitions = 128 m-rows; free dims are s-major (s, n) so the
dispersion shift n -> n+l is a free-dim offset, the broadcast of x over s is a
stride-0 outer AP dim (dense innermost keeps DVE 2x mode), and the shot-sum
becomes contiguous stripe-halving adds.  Pipeline is fp16 (~1e-3 max rel err
vs fp64, validated).  Per band l:
  stage 1: h_l = sum_c F[c,l]*a_c (ScalarE seeds + partials, DVE/GpSimd adds),
           Y[:, l:l+N] += h_l * x[:, l-bcast]  (DVE), h_l spilled to DRAM
  stage 2: h_l reloaded (DMA, hidden), t = h_l * Y[:, l:l+N] (DVE),
           X[:, l] = stripe-tree shot sum (GpSimd first level, DVE rest)
"""

import numpy as np

B, M, N, L, S = 4, 256, 256, 24, 22
MSH = M // 2                     # rows per core
NCORES = 8
NS, NL = N * S, N * L
NP = N + L - 1                   # 279 shifted columns
YW = NP * S                      # Y free width (s-major: s outer, n' inner)


def _bases() -> np.ndarray:
    """(4, L) color responses paired row-wise with (wr, wg, wb, wc)."""
    wl = np.linspace(400.0, 700.0, L)

    def g(mu: float, sig: float) -> np.ndarray:
        return np.exp(-0.5 * ((wl - mu) / sig) ** 2)

    # reference: H = wr*f620 + wg*f550 + wb*f450 + wc*f500 (fr,fg,fc,fb = 620,550,500,450)
    return np.stack([g(620.0, 50.0), g(550.0, 50.0), g(450.0, 50.0), g(500.0, 50.0)])


_NC = None


def _build():
    import concourse.bacc as bacc
    import concourse.mybir as mybir
    import concourse.tile as tile

    f16, f32 = mybir.dt.float16, mybir.dt.float32
    A = mybir.AluOpType
    F = _bases()

    nc = bacc.Bacc("TRN2", target_bir_lowering=False, debug=False, num_devices=NCORES)
    xin = nc.declare_dram_parameter("x16", [MSH, NL], f16, isOutput=False)   # (l, n)
    wins = [
        nc.declare_dram_parameter(f"w{i}", [MSH, NS], f16, isOutput=False)   # (s, n)
        for i in range(4)
    ]
    out = nc.declare_dram_parameter("out", [MSH, NL], f32, isOutput=True)    # (l, n)
    hcache = nc.dram_tensor("hcache", [L, MSH, NS], f16)

    with tile.TileContext(nc) as tc:
        with (
            tc.tile_pool(name="main", bufs=1) as main,
            tc.tile_pool(name="hp", bufs=3) as hp,
            tc.tile_pool(name="tp", bufs=4) as tp,
            tc.tile_pool(name="pp", bufs=2) as pp,
        ):
            a = [main.tile([MSH, NS], f16, tag=f"a{i}", name=f"a{i}") for i in range(4)]
            xt = main.tile([MSH, NL], f16, tag="x", bufs=2, name="xt")
            Y = main.tile([MSH, YW], f16, tag="Y", name="Yt")

            for i in range(4):
                nc.sync.dma_start(a[i][:], wins[i][:])
            nc.sync.dma_start(xt[:], xin[:])
            nc.gpsimd.memset(Y[:], 0.0)

            # a_c = w_c / (wr+wg+wb+wc)
            u = hp.tile([MSH, NS], f16, tag="h", name="ut")
            nc.vector.tensor_tensor(u[:], a[0][:], a[1][:], A.add)
            nc.vector.tensor_tensor(u[:], u[:], a[2][:], A.add)
            nc.vector.tensor_tensor(u[:], u[:], a[3][:], A.add)
            with nc.allow_low_precision("fp16 pipeline, validated ~1e-3 vs fp64"):
                nc.vector.reciprocal(u[:], u[:])
            for i in range(4):
                nc.vector.tensor_tensor(a[i][:], a[i][:], u[:], A.mult)

            x3 = xt[:].rearrange("p (l n) -> p l n", n=N)
            Y3 = Y[:].rearrange("p (s n) -> p s n", n=NP)

            # Stage 1: Y[:, s, l+n] += h_l[:, s, n] * x[:, l, n];  h_l -> DRAM
            for l in range(L):
                h = hp.tile([MSH, NS], f16, tag="h", name="ht")
                t1 = tp.tile([MSH, NS], f16, tag="tp", name="t1t")
                t2 = tp.tile([MSH, NS], f16, tag="tp", name="t2t")
                t3 = tp.tile([MSH, NS], f16, tag="tp", name="t3t")
                nc.scalar.mul(h[:], a[0][:], float(F[0, l]))          # ACT seed
                nc.vector.tensor_scalar_mul(t1[:], a[1][:], float(F[1, l]))
                nc.scalar.mul(t2[:], a[2][:], float(F[2, l]))         # ACT partial
                half = NS // 2                                        # ACT/DVE split
                nc.scalar.mul(t3[:, :half], a[3][:, :half], float(F[3, l]))
                nc.vector.tensor_scalar_mul(t3[:, half:], a[3][:, half:], float(F[3, l]))
                nc.vector.tensor_tensor(h[:], h[:], t1[:], A.add)
                nc.vector.tensor_tensor(h[:], h[:], t2[:], A.add)
                nc.vector.tensor_tensor(h[:], h[:], t3[:], A.add)
                nc.sync.dma_start(hcache[l], h[:])
                p = pp.tile([MSH, NS], f16, tag="p", name="pt")
                xb = x3[:, l, :].unsqueeze(1).broadcast_to((MSH, S, N))
                nc.vector.tensor_tensor(
                    p[:].rearrange("p (s n) -> p s n", n=N),
                    h[:].rearrange("p (s n) -> p s n", n=N),
                    xb,
                    A.mult,
                )
                ysl = Y3[:, :, l : l + N]
                nc.gpsimd.tensor_tensor(                              # GpSimd acc
                    ysl, ysl, p[:].rearrange("p (s n) -> p s n", n=N), A.add
                )

            # Stage 2: X[:, l, n] = sum_s h_l[:, s, n] * Y[:, s, l+n]
            for l in range(L):
                h = main.tile([MSH, NL], f16, tag="x", bufs=2, name="hin")
                nc.sync.dma_start(h[:, :NS], hcache[l])
                t = pp.tile([MSH, NS], f16, tag="p", name="tt")
                nc.vector.tensor_tensor(
                    t[:].rearrange("p (s n) -> p s n", n=N),
                    h[:, :NS].rearrange("p (s n) -> p s n", n=N),
                    Y3[:, :, l : l + N],
                    A.mult,
                )
                # shot-sum tree over 22 contiguous stripes of N
                tv = t[:]
                nc.gpsimd.tensor_tensor(
                    tv[:, : 11 * N], tv[:, : 11 * N], tv[:, 11 * N : 22 * N], A.add
                )
                nc.vector.tensor_tensor(
                    tv[:, : 5 * N], tv[:, : 5 * N], tv[:, 5 * N : 10 * N], A.add
                )
                nc.vector.tensor_tensor(
                    tv[:, : 2 * N], tv[:, : 2 * N], tv[:, 2 * N : 4 * N], A.add
                )
                nc.vector.tensor_tensor(tv[:, :N], tv[:, :N], tv[:, N : 2 * N], A.add)
                nc.vector.tensor_tensor(
                    tv[:, :N], tv[:, :N], tv[:, 4 * N : 5 * N], A.add
                )
                xol = tp.tile([MSH, N], f32, tag="xol", bufs=2, name="xolt")
                nc.vector.tensor_tensor(
                    xol[:], tv[:, :N], tv[:, 10 * N : 11 * N], A.add
                )
                nc.sync.dma_start(out[:, l * N : (l + 1) * N], xol[:])

    nc.compile()
    return nc


def _get_nc():
    global _NC
    if _NC is None:
        _NC = _build()
    return _NC


def _make_in_maps(x, wr, wg, wb, wc):
    x = np.asarray(x, dtype=np.float32)
    ws = [np.asarray(w, dtype=np.float32).reshape(M, M, S) for w in (wr, wg, wb, wc)]
    in_maps = []
    for core in range(NCORES):
        b, mh = divmod(core, 2)
        rows = slice(mh * MSH, (mh + 1) * MSH)
        xs = x[b, rows].transpose(0, 2, 1)            # (MSH, L, N)
        m = {"x16": np.ascontiguousarray(xs).reshape(MSH, NL).astype(np.float16)}
        for i, w in enumerate(ws):
            wsb = w[rows].transpose(0, 2, 1)          # (MSH, S, N)
            m[f"w{i}"] = np.ascontiguousarray(wsb).reshape(MSH, NS).astype(np.float16)
        in_maps.append(m)
    return in_maps


def _run_shards(in_maps):
    from concourse.bass_utils import run_bass_kernel_spmd

    nc = _get_nc()
    return run_bass_kernel_spmd(nc, in_maps, list(range(NCORES)))


def kernel(x, wr, wg, wb, wc):
    res = _run_shards(_make_in_maps(x, wr, wg, wb, wc))
    X = np.empty((B, M, N, L), dtype=np.float32)
    for core in range(NCORES):
        b, mh = divmod(core, 2)
        xo = res.results[core]["out"].reshape(MSH, L, N).transpose(0, 2, 1)
        X[b, mh * MSH : (mh + 1) * MSH] = xo
    return X / X.max()


def estimate_ns() -> float:
    """Single-core cost-model estimate of the kernel duration (ns)."""
    from concourse.timeline_sim import TimelineSim

    return TimelineSim(_get_nc()).simulate()


# revision 14
# speedup vs baseline: 2.1929x; 1.0204x over previous
"""CASSI colored-aperture layer (nn_CASSI_layer_Colored) on 8 Trainium2 NeuronCores.

Reference semantics (B=4, M=N=KERN=256, L=24 bands, S=22 shots):
    H[m,n,l,s] = (wr*fr[l] + wg*fg[l] + wb*fb[l] + wc*fc[l]) / (wr+wg+wb+wc)
    Y[b,m,n',s] = sum_l H[m,n'-l,l,s] * x[b,m,n'-l,l]          (dispersion shift-sum)
    X[b,m,n,l]  = sum_s H[m,n,l,s] * Y[b,m,n+l,s]              (adjoint + shot sum)
    out = X / max(X)

Sharding: data-parallel over (batch b, row-half mh): 4 x 2 = 8 cores.  Rows m
never couple, so each core computes 128 rows of one batch independently; only
the final global max couples shards (host side, after the gather).

Per-core mapping: partitions = 128 m-rows; free dims are s-major (s, n) so the
dispersion shift n -> n+l is a free-dim offset, the broadcast of x over s is a
stride-0 outer AP dim (dense innermost keeps DVE 2x mode), and the shot-sum
becomes contiguous stripe-halving adds.  Pipeline is fp16 (~1e-3 max rel err
vs fp64, validated).  Per band l:
  stage 1: h_l = sum_c F[c,l]*a_c (ScalarE seeds + partials, DVE/GpSimd adds),
           Y[:, l:l+N] += h_l * x[:, l-bcast]  (DVE), h_l spilled to DRAM
  stage 2: h_l reloaded (DMA, hidden), t = h_l * Y[:, l:l+N] (DVE),
           X[:, l] = stripe-tree shot sum (GpSimd first level, DVE rest)
"""

import numpy as np

B, M, N, L, S = 4, 256, 256, 24, 22
MSH = M // 2                     # rows per core
NCORES = 8
NS, NL = N * S, N * L
NP = N + L - 1                   # 279 shifted columns
YW = NP * S                      # Y free width (s-major: s outer, n' inner)


def _bases() -> np.ndarray:
    """(4, L) color responses paired row-wise with (wr, wg, wb, wc)."""
    wl = np.linspace(400.0, 700.0, L)

    def g(mu: float, sig: float) -> np.ndarray:
        return np.exp(-0.5 * ((wl - mu) / sig) ** 2)

    # reference: H = wr*f620 + wg*f550 + wb*f450 + wc*f500 (fr,fg,fc,fb = 620,550,500,450)
    return np.stack([g(620.0, 50.0), g(550.0, 50.0), g(450.0, 50.0), g(500.0, 50.0)])


_NC = None


def _build():
    import concourse.bacc as bacc
    import concourse.mybir as mybir
    import concourse.tile as tile

    f16, f32 = mybir.dt.float16, mybir.dt.float32
    A = mybir.AluOpType
    F = _bases()

    nc = bacc.Bacc("TRN2", target_bir_lowering=False, debug=False, num_devices=NCORES)
    xin = nc.declare_dram_parameter("x16", [MSH, NL], f16, isOutput=False)   # (l, n)
    wins = [
        nc.declare_dram_parameter(f"w{i}", [MSH, NS], f16, isOutput=False)   # (s, n)
        for i in range(4)
    ]
    out = nc.declare_dram_parameter("out", [MSH, NL], f32, isOutput=True)    # (l, n)
    hcache = nc.dram_tensor("hcache", [L, MSH, NS], f16)

    with tile.TileContext(nc) as tc:
        with (
            tc.tile_pool(name="main", bufs=1) as main,
            tc.tile_pool(name="hp", bufs=3) as hp,
            tc.tile_pool(name="tp", bufs=4) as tp,
            tc.tile_pool(name="pp", bufs=2) as pp,
        ):
            a = [main.tile([MSH, NS], f16, tag=f"a{i}", name=f"a{i}") for i in range(4)]
            xt = main.tile([MSH, NL], f16, tag="x", bufs=2, name="xt")
            Y = main.tile([MSH, YW], f16, tag="Y", name="Yt")

            for i in range(4):
                nc.sync.dma_start(a[i][:], wins[i][:])
            nc.sync.dma_start(xt[:], xin[:])
            nc.gpsimd.memset(Y[:], 0.0)

            # a_c = w_c / (wr+wg+wb+wc)
            u = hp.tile([MSH, NS], f16, tag="h", name="ut")
            nc.vector.tensor_tensor(u[:], a[0][:], a[1][:], A.add)
            nc.vector.tensor_tensor(u[:], u[:], a[2][:], A.add)
            nc.vector.tensor_tensor(u[:], u[:], a[3][:], A.add)
            with nc.allow_low_precision("fp16 pipeline, validated ~1e-3 vs fp64"):
                nc.vector.reciprocal(u[:], u[:])
            for i in range(3):
                nc.vector.tensor_tensor(a[i][:], a[i][:], u[:], A.mult)
            nc.gpsimd.tensor_tensor(a[3][:], a[3][:], u[:], A.mult)

            x3 = xt[:].rearrange("p (l n) -> p l n", n=N)
            Y3 = Y[:].rearrange("p (s n) -> p s n", n=NP)

            # Stage 1: Y[:, s, l+n] += h_l[:, s, n] * x[:, l, n];  h_l -> DRAM
            for l in range(L):
                h = hp.tile([MSH, NS], f16, tag="h", name="ht")
                t1 = tp.tile([MSH, NS], f16, tag="tp", name="t1t")
                t2 = tp.tile([MSH, NS], f16, tag="tp", name="t2t")
                t3 = tp.tile([MSH, NS], f16, tag="tp", name="t3t")
                nc.scalar.mul(h[:], a[0][:], float(F[0, l]))          # ACT seed
                cut = (NS * 3) // 4                                   # ACT/DVE balance
                nc.scalar.mul(t1[:, :cut], a[1][:, :cut], float(F[1, l]))
                nc.vector.tensor_scalar_mul(t1[:, cut:], a[1][:, cut:], float(F[1, l]))
                nc.scalar.mul(t2[:], a[2][:], float(F[2, l]))         # ACT partial
                nc.vector.tensor_scalar_mul(t3[:], a[3][:], float(F[3, l]))
                nc.vector.tensor_tensor(h[:], h[:], t1[:], A.add)
                nc.vector.tensor_tensor(h[:], h[:], t2[:], A.add)
                nc.vector.tensor_tensor(h[:], h[:], t3[:], A.add)
                nc.sync.dma_start(hcache[l], h[:])
                p = pp.tile([MSH, NS], f16, tag="p", name="pt")
                xb = x3[:, l, :].unsqueeze(1).broadcast_to((MSH, S, N))
                nc.vector.tensor_tensor(
                    p[:].rearrange("p (s n) -> p s n", n=N),
                    h[:].rearrange("p (s n) -> p s n", n=N),
                    xb,
                    A.mult,
                )
                ysl = Y3[:, :, l : l + N]
                nc.gpsimd.tensor_tensor(                              # GpSimd acc
                    ysl, ysl, p[:].rearrange("p (s n) -> p s n", n=N), A.add
                )

            # Stage 2: X[:, l, n] = sum_s h_l[:, s, n] * Y[:, s, l+n]
            for l in range(L):
                h = main.tile([MSH, NL], f16, tag="x", bufs=2, name="hin")
                nc.sync.dma_start(h[:, :NS], hcache[l])
                t = pp.tile([MSH, NS], f16, tag="p", name="tt")
                nc.vector.tensor_tensor(
                    t[:].rearrange("p (s n) -> p s n", n=N),
                    h[:, :NS].rearrange("p (s n) -> p s n", n=N),
                    Y3[:, :, l : l + N],
                    A.mult,
                )
                # shot-sum tree over 22 contiguous stripes of N
                tv = t[:]
                nc.vector.tensor_tensor(
                    tv[:, : 11 * N], tv[:, : 11 * N], tv[:, 11 * N : 22 * N], A.add
                )
                nc.gpsimd.tensor_tensor(
                    tv[:, : 5 * N], tv[:, : 5 * N], tv[:, 5 * N : 10 * N], A.add
                )
                nc.vector.tensor_tensor(
                    tv[:, : 2 * N], tv[:, : 2 * N], tv[:, 2 * N : 4 * N], A.add
                )
                nc.vector.tensor_tensor(tv[:, :N], tv[:, :N], tv[:, N : 2 * N], A.add)
                nc.vector.tensor_tensor(
                    tv[:, :N], tv[:, :N], tv[:, 4 * N : 5 * N], A.add
                )
                xol = tp.tile([MSH, N], f32, tag="xol", bufs=2, name="xolt")
                nc.vector.tensor_tensor(
                    xol[:], tv[:, :N], tv[:, 10 * N : 11 * N], A.add
                )
                nc.sync.dma_start(out[:, l * N : (l + 1) * N], xol[:])

    nc.compile()
    return nc


def _get_nc():
    global _NC
    if _NC is None:
        _NC = _build()
    return _NC


def _make_in_maps(x, wr, wg, wb, wc):
    x = np.asarray(x, dtype=np.float32)
    ws = [np.asarray(w, dtype=np.float32).reshape(M, M, S) for w in (wr, wg, wb, wc)]
    in_maps = []
    for core in range(NCORES):
        b, mh = divmod(core, 2)
        rows = slice(mh * MSH, (mh + 1) * MSH)
        xs = x[b, rows].transpose(0, 2, 1)            # (MSH, L, N)
        m = {"x16": np.ascontiguousarray(xs).reshape(MSH, NL).astype(np.float16)}
        for i, w in enumerate(ws):
            wsb = w[rows].transpose(0, 2, 1)          # (MSH, S, N)
            m[f"w{i}"] = np.ascontiguousarray(wsb).reshape(MSH, NS).astype(np.float16)
        in_maps.append(m)
    return in_maps


def _run_shards(in_maps):
    from concourse.bass_utils import run_bass_kernel_spmd

    nc = _get_nc()
    return run_bass_kernel_spmd(nc, in_maps, list(range(NCORES)))


def kernel(x, wr, wg, wb, wc):
    res = _run_shards(_make_in_maps(x, wr, wg, wb, wc))
    X = np.empty((B, M, N, L), dtype=np.float32)
    for core in range(NCORES):
        b, mh = divmod(core, 2)
        xo = res.results[core]["out"].reshape(MSH, L, N).transpose(0, 2, 1)
        X[b, mh * MSH : (mh + 1) * MSH] = xo
    return X / X.max()


def estimate_ns() -> float:
    """Single-core cost-model estimate of the kernel duration (ns)."""
    from concourse.timeline_sim import TimelineSim

    return TimelineSim(_get_nc()).simulate()
